# revision 1
# baseline (speedup 1.0000x reference)
"""Trainium2 Bass kernel for CustomMLP: out = GELU(x@W1+b1)@W2 + b2.

x: (4, 2048, 1024) f32, W1: (1024, 4096), b1: (4096,), W2: (4096, 1024),
b2: (1024,). Data-parallel over the 8192 flattened rows: each of the 8
NeuronCores handles 1024 rows with fully replicated weights (no
collectives).

Per-core layout (everything transposed so both matmuls contract on the
partition axis with no on-chip transposes):
  xT   [1024(e), 1024(m)]           = x_shard^T
  hT   [h, m] computed on chip      (GELU applied on PSUM eviction)
  outT [1024(e2), 1024(m)]          host transposes back

matmul1: psum[h_blk, m] += w1[e_blk, h_blk].T @ xT[e_blk, m]
matmul2: psum[e2_blk, m] += w2[h_blk, e2_blk].T @ hT[h_blk, m]

Weights are host-packed so every DMA lands 4KB-contiguous per partition.
Matmuls run as float32r (full fp32 storage; 1 PE cycle/row at N>=256).
"""
import numpy as np

import concourse.bass as bass
import concourse.mybir as mybir
import concourse.tile as tile
from concourse import bacc
from concourse.bass_utils import run_bass_kernel_spmd

P = 128
N_CORES = 8

F32 = mybir.dt.float32
F32R = mybir.dt.float32r
GELU = mybir.ActivationFunctionType.Gelu
IDENT = mybir.ActivationFunctionType.Identity


def build_nc(M=1024, E=1024, H=4096, E2=1024, mm_dtype=F32R, act=GELU):
    """Build + compile the per-core program. M/E/H/E2 parameterized so a
    scaled-down version can run in CoreSim."""
    EB, HB, E2B = E // P, H // P, E2 // P
    MH = max(1, M // 512)  # m halves (moving-dim chunks of <=512)
    MS = M // MH           # moving chunk size
    W2Q = min(8, HB)       # h-blocks per w2 slab
    NQ = HB // W2Q         # w2 slabs per e2 block

    mmdt = mm_dtype
    nc = bacc.Bacc(None, target_bir_lowering=False)
    xT_d = nc.declare_dram_parameter("xT", [E, M], mmdt, isOutput=False)
    w1_d = nc.declare_dram_parameter("w1p", [HB, P, EB, P], mmdt, isOutput=False)
    b1_d = nc.declare_dram_parameter("b1p", [P, HB], F32, isOutput=False)
    w2_d = nc.declare_dram_parameter("w2p", [E2B, P, HB, P], mmdt, isOutput=False)
    b2_d = nc.declare_dram_parameter("b2p", [P, E2B], F32, isOutput=False)
    out_d = nc.declare_dram_parameter("outT", [E2B, P, M], F32, isOutput=True)

    xT_v = xT_d.rearrange("(eb p) m -> p eb m", p=P)

    with tile.TileContext(nc) as tc:
        with (
            tc.tile_pool(name="const", bufs=1) as cpool,
            tc.tile_pool(name="xp", bufs=1) as xpool,
            tc.tile_pool(name="hp", bufs=1) as hpool,
            tc.tile_pool(name="w1p", bufs=5) as w1pool,
            tc.tile_pool(name="w2p", bufs=4) as w2pool,
            tc.tile_pool(name="op", bufs=2) as opool,
            tc.tile_pool(name="ps1", bufs=3, space="PSUM") as psum1,
            tc.tile_pool(name="ps2", bufs=3, space="PSUM") as psum2,
        ):
            b1_sb = cpool.tile([P, HB], F32, name="b1s")
            b2_sb = cpool.tile([P, E2B], F32, name="b2s")
            nc.sync.dma_start(out=b1_sb[:], in_=b1_d[:])
            nc.sync.dma_start(out=b2_sb[:], in_=b2_d[:])

            # DMA queue is one FIFO: emit in consumption order. The first
            # HEAD_HBS h-blocks run m-half-major (all mh0 groups, then mh1)
            # so the PE's early xT demand rate is halved while the queue
            # ramps; w1 slabs for those blocks stay live across both halves.
            HEAD_HBS = min(4, HB)
            w1_tiles = {}
            w1_tiles[0] = w1pool.tile([P, EB, P], mmdt, name="w1t")
            nc.sync.dma_start(out=w1_tiles[0][:], in_=w1_d[0])

            xT_sb = xpool.tile([P, EB, M], mmdt, name="xT")
            for eb in range(EB):
                nc.sync.dma_start(out=xT_sb[:, eb, 0:MS], in_=xT_v[:, eb, 0:MS])
            for hb in range(1, HEAD_HBS):
                w1_tiles[hb] = w1pool.tile([P, EB, P], mmdt, name="w1t")
                nc.sync.dma_start(out=w1_tiles[hb][:], in_=w1_d[hb])
            for mh in range(1, MH):
                ms = slice(mh * MS, (mh + 1) * MS)
                for eb in range(EB):
                    nc.sync.dma_start(out=xT_sb[:, eb, ms], in_=xT_v[:, eb, ms])

            hT_sb = hpool.tile([P, HB, M], mmdt, name="hT")

            def mm1_group(w1_t, hb, mh):
                ms = slice(mh * MS, (mh + 1) * MS)
                ps = psum1.tile([P, MS], F32, name="ps1")
                for eb in range(EB):
                    nc.tensor.matmul(
                        ps[:],
                        lhsT=w1_t[:, eb, :],
                        rhs=xT_sb[:, eb, ms],
                        start=(eb == 0),
                        stop=(eb == EB - 1),
                    )
                nc.scalar.activation(
                    hT_sb[:, hb, ms], ps[:], act, bias=b1_sb[:, hb : hb + 1]
                )

            # ---- matmul 1 + GELU ----
            for mh in range(MH):
                for hb in range(HEAD_HBS):
                    mm1_group(w1_tiles[hb], hb, mh)
            for hb in range(HEAD_HBS, HB):
                w1_t = w1pool.tile([P, EB, P], mmdt, name="w1t")
                nc.sync.dma_start(out=w1_t[:], in_=w1_d[hb])
                for mh in range(MH):
                    mm1_group(w1_t, hb, mh)

            # ---- matmul 2 + bias ----
            for e2b in range(E2B):
                w2_ts = []
                for q in range(NQ):
                    w2_t = w2pool.tile([P, W2Q, P], mmdt, name="w2t")
                    nc.sync.dma_start(
                        out=w2_t[:], in_=w2_d[e2b, :, q * W2Q : (q + 1) * W2Q, :]
                    )
                    w2_ts.append(w2_t)
                out_sb = opool.tile([P, M], F32, name="outsb")
                for mh in range(MH):
                    ms = slice(mh * MS, (mh + 1) * MS)
                    ps2 = psum2.tile([P, MS], F32, name="ps2")
                    for hb in range(HB):
                        nc.tensor.matmul(
                            ps2[:],
                            lhsT=w2_ts[hb // W2Q][:, hb % W2Q, :],
                            rhs=hT_sb[:, hb, ms],
                            start=(hb == 0),
                            stop=(hb == HB - 1),
                        )
                    nc.scalar.activation(
                        out_sb[:, ms], ps2[:], IDENT, bias=b2_sb[:, e2b : e2b + 1]
                    )
                    nc.sync.dma_start(out=out_d[e2b, :, ms], in_=out_sb[:, ms])

    nc.compile()
    return nc


def pack_inputs(x, w1, b1, w2, b2):
    """Host-side shard + pack. Returns per-core input maps."""
    M_TOT = x.shape[0] * x.shape[1]
    E = x.shape[2]
    H = w1.shape[1]
    E2 = w2.shape[1]
    MC = M_TOT // N_CORES
    xf = np.ascontiguousarray(x.reshape(M_TOT, E))

    w1p = np.ascontiguousarray(
        w1.reshape(E // P, P, H // P, P).transpose(2, 1, 0, 3)
    )
    w2p = np.ascontiguousarray(
        w2.reshape(H // P, P, E2 // P, P).transpose(2, 1, 0, 3)
    )
    b1p = np.ascontiguousarray(b1.reshape(H // P, P).T)
    b2p = np.ascontiguousarray(b2.reshape(E2 // P, P).T)

    in_maps = []
    for i in range(N_CORES):
        xTi = np.ascontiguousarray(xf[i * MC : (i + 1) * MC].T)
        in_maps.append(
            {"xT": xTi, "w1p": w1p, "b1p": b1p, "w2p": w2p, "b2p": b2p}
        )
    return in_maps


def unpack_outputs(results, batch_shape=(4, 2048), E2=1024):
    M_TOT = batch_shape[0] * batch_shape[1]
    MC = M_TOT // N_CORES
    out = np.empty((M_TOT, E2), dtype=np.float32)
    for i in range(N_CORES):
        o = results[i]["outT"]  # [E2B, P, MC]
        out[i * MC : (i + 1) * MC] = o.transpose(2, 0, 1).reshape(MC, E2)
    return out.reshape(*batch_shape, E2)


_NC_CACHE = {}


def _get_nc():
    if "nc" not in _NC_CACHE:
        _NC_CACHE["nc"] = build_nc()
    return _NC_CACHE["nc"]


def kernel(x, w1, b1, w2, b2):
    nc = _get_nc()
    in_maps = pack_inputs(
        np.asarray(x, dtype=np.float32),
        np.asarray(w1, dtype=np.float32),
        np.asarray(b1, dtype=np.float32),
        np.asarray(w2, dtype=np.float32),
        np.asarray(b2, dtype=np.float32),
    )
    res = run_bass_kernel_spmd(nc, in_maps, core_ids=list(range(N_CORES))).results
    return unpack_outputs(res, batch_shape=(x.shape[0], x.shape[1]), E2=w2.shape[1])



# revision 3
# speedup vs baseline: 1.0874x; 1.0874x over previous
"""Trainium2 Bass kernel for CustomMLP: out = GELU(x@W1+b1)@W2 + b2.

x: (4, 2048, 1024) f32, W1: (1024, 4096), b1: (4096,), W2: (4096, 1024),
b2: (1024,). Data-parallel over the 8192 flattened rows: each of the 8
NeuronCores handles 1024 rows with fully replicated weights (no
collectives).

Per-core layout (everything transposed so both matmuls contract on the
partition axis with no on-chip transposes):
  xT   [1024(e), 1024(m)]           = x_shard^T
  hT   [h, m] computed on chip      (GELU applied on PSUM eviction)
  outT [1024(e2), 1024(m)]          host transposes back

matmul1: psum[h_blk, m] += w1[e_blk, h_blk].T @ xT[e_blk, m]
matmul2: psum[e2_blk, m] += w2[h_blk, e2_blk].T @ hT[h_blk, m]

Weights are host-packed so every DMA lands 4KB-contiguous per partition.
Matmuls run as float32r (full fp32 storage; 1 PE cycle/row at N>=256).
"""
import ml_dtypes
import numpy as np

import concourse.bass as bass
import concourse.mybir as mybir
import concourse.tile as tile
from concourse import bacc
from concourse.bass_utils import run_bass_kernel_spmd

P = 128
N_CORES = 8

F32 = mybir.dt.float32
F32R = mybir.dt.float32r
BF16 = mybir.dt.bfloat16
NP_BF16 = ml_dtypes.bfloat16
GELU = mybir.ActivationFunctionType.Gelu
IDENT = mybir.ActivationFunctionType.Identity


def build_nc(M=1024, E=1024, H=4096, E2=1024, mm_dtype=BF16, act=GELU):
    """Build + compile the per-core program. M/E/H/E2 parameterized so a
    scaled-down version can run in CoreSim."""
    EB, HB, E2B = E // P, H // P, E2 // P
    MH = max(1, M // 512)  # m halves (moving-dim chunks of <=512)
    MS = M // MH           # moving chunk size
    W2Q = min(8, HB)       # h-blocks per w2 slab
    NQ = HB // W2Q         # w2 slabs per e2 block

    mmdt = mm_dtype
    nc = bacc.Bacc(None, target_bir_lowering=False)
    xT_d = nc.declare_dram_parameter("xT", [E, M], mmdt, isOutput=False)
    w1_d = nc.declare_dram_parameter("w1p", [HB, P, EB, P], mmdt, isOutput=False)
    b1_d = nc.declare_dram_parameter("b1p", [P, HB], F32, isOutput=False)
    w2_d = nc.declare_dram_parameter("w2p", [E2B, P, HB, P], mmdt, isOutput=False)
    b2_d = nc.declare_dram_parameter("b2p", [P, E2B], F32, isOutput=False)
    out_d = nc.declare_dram_parameter("outT", [E2B, P, M], F32, isOutput=True)

    xT_v = xT_d.rearrange("(eb p) m -> p eb m", p=P)

    with tile.TileContext(nc) as tc:
        with (
            tc.tile_pool(name="const", bufs=1) as cpool,
            tc.tile_pool(name="xp", bufs=1) as xpool,
            tc.tile_pool(name="hp", bufs=1) as hpool,
            tc.tile_pool(name="w1p", bufs=5) as w1pool,
            tc.tile_pool(name="w2p", bufs=4) as w2pool,
            tc.tile_pool(name="op", bufs=2) as opool,
            tc.tile_pool(name="ps1", bufs=3, space="PSUM") as psum1,
            tc.tile_pool(name="ps2", bufs=3, space="PSUM") as psum2,
        ):
            b1_sb = cpool.tile([P, HB], F32, name="b1s")
            b2_sb = cpool.tile([P, E2B], F32, name="b2s")
            nc.sync.dma_start(out=b1_sb[:], in_=b1_d[:])
            nc.sync.dma_start(out=b2_sb[:], in_=b2_d[:])

            # DMA queue is one FIFO: emit in consumption order. The first
            # HEAD_HBS h-blocks run m-half-major (all mh0 groups, then mh1)
            # so the PE's early xT demand rate is halved while the queue
            # ramps; w1 slabs for those blocks stay live across both halves.
            HEAD_HBS = min(4, HB)
            w1_tiles = {}
            w1_tiles[0] = w1pool.tile([P, EB, P], mmdt, name="w1t")
            nc.sync.dma_start(out=w1_tiles[0][:], in_=w1_d[0])

            xT_sb = xpool.tile([P, EB, M], mmdt, name="xT")
            for eb in range(EB):
                nc.sync.dma_start(out=xT_sb[:, eb, 0:MS], in_=xT_v[:, eb, 0:MS])
            for hb in range(1, HEAD_HBS):
                w1_tiles[hb] = w1pool.tile([P, EB, P], mmdt, name="w1t")
                nc.sync.dma_start(out=w1_tiles[hb][:], in_=w1_d[hb])
            for mh in range(1, MH):
                ms = slice(mh * MS, (mh + 1) * MS)
                for eb in range(EB):
                    nc.sync.dma_start(out=xT_sb[:, eb, ms], in_=xT_v[:, eb, ms])

            hT_sb = hpool.tile([P, HB, M], mmdt, name="hT")

            def mm1_group(w1_t, hb, mh):
                ms = slice(mh * MS, (mh + 1) * MS)
                ps = psum1.tile([P, MS], F32, name="ps1")
                for eb in range(EB):
                    nc.tensor.matmul(
                        ps[:],
                        lhsT=w1_t[:, eb, :],
                        rhs=xT_sb[:, eb, ms],
                        start=(eb == 0),
                        stop=(eb == EB - 1),
                    )
                nc.scalar.activation(
                    hT_sb[:, hb, ms], ps[:], act, bias=b1_sb[:, hb : hb + 1]
                )

            # ---- matmul 1 + GELU ----
            for mh in range(MH):
                for hb in range(HEAD_HBS):
                    mm1_group(w1_tiles[hb], hb, mh)
            for hb in range(HEAD_HBS, HB):
                w1_t = w1pool.tile([P, EB, P], mmdt, name="w1t")
                nc.sync.dma_start(out=w1_t[:], in_=w1_d[hb])
                for mh in range(MH):
                    mm1_group(w1_t, hb, mh)

            # ---- matmul 2 + bias ----
            for e2b in range(E2B):
                w2_ts = []
                for q in range(NQ):
                    w2_t = w2pool.tile([P, W2Q, P], mmdt, name="w2t")
                    nc.sync.dma_start(
                        out=w2_t[:], in_=w2_d[e2b, :, q * W2Q : (q + 1) * W2Q, :]
                    )
                    w2_ts.append(w2_t)
                out_sb = opool.tile([P, M], F32, name="outsb")
                for mh in range(MH):
                    ms = slice(mh * MS, (mh + 1) * MS)
                    ps2 = psum2.tile([P, MS], F32, name="ps2")
                    for hb in range(HB):
                        nc.tensor.matmul(
                            ps2[:],
                            lhsT=w2_ts[hb // W2Q][:, hb % W2Q, :],
                            rhs=hT_sb[:, hb, ms],
                            start=(hb == 0),
                            stop=(hb == HB - 1),
                        )
                    nc.scalar.activation(
                        out_sb[:, ms], ps2[:], IDENT, bias=b2_sb[:, e2b : e2b + 1]
                    )
                    nc.sync.dma_start(out=out_d[e2b, :, ms], in_=out_sb[:, ms])

    nc.compile()
    return nc


def pack_inputs(x, w1, b1, w2, b2):
    """Host-side shard + pack. Returns per-core input maps."""
    M_TOT = x.shape[0] * x.shape[1]
    E = x.shape[2]
    H = w1.shape[1]
    E2 = w2.shape[1]
    MC = M_TOT // N_CORES
    xf = np.ascontiguousarray(x.reshape(M_TOT, E))

    w1p = np.ascontiguousarray(
        w1.reshape(E // P, P, H // P, P).transpose(2, 1, 0, 3)
    ).astype(NP_BF16)
    w2p = np.ascontiguousarray(
        w2.reshape(H // P, P, E2 // P, P).transpose(2, 1, 0, 3)
    ).astype(NP_BF16)
    b1p = np.ascontiguousarray(b1.reshape(H // P, P).T)
    b2p = np.ascontiguousarray(b2.reshape(E2 // P, P).T)

    in_maps = []
    for i in range(N_CORES):
        xTi = np.ascontiguousarray(xf[i * MC : (i + 1) * MC].T).astype(NP_BF16)
        in_maps.append(
            {"xT": xTi, "w1p": w1p, "b1p": b1p, "w2p": w2p, "b2p": b2p}
        )
    return in_maps


def unpack_outputs(results, batch_shape=(4, 2048), E2=1024):
    M_TOT = batch_shape[0] * batch_shape[1]
    MC = M_TOT // N_CORES
    out = np.empty((M_TOT, E2), dtype=np.float32)
    for i in range(N_CORES):
        o = results[i]["outT"]  # [E2B, P, MC]
        out[i * MC : (i + 1) * MC] = o.transpose(2, 0, 1).reshape(MC, E2)
    return out.reshape(*batch_shape, E2)


_NC_CACHE = {}


def _get_nc():
    if "nc" not in _NC_CACHE:
        _NC_CACHE["nc"] = build_nc()
    return _NC_CACHE["nc"]


def kernel(x, w1, b1, w2, b2):
    nc = _get_nc()
    in_maps = pack_inputs(
        np.asarray(x, dtype=np.float32),
        np.asarray(w1, dtype=np.float32),
        np.asarray(b1, dtype=np.float32),
        np.asarray(w2, dtype=np.float32),
        np.asarray(b2, dtype=np.float32),
    )
    res = run_bass_kernel_spmd(nc, in_maps, core_ids=list(range(N_CORES))).results
    return unpack_outputs(res, batch_shape=(x.shape[0], x.shape[1]), E2=w2.shape[1])

